# revision 1
# baseline (speedup 1.0000x reference)
"""Tensor-parallel causal-self-attention (full-attention) Bass kernel for TRN2.

Sharding: 16 heads over 8 cores (2 heads/core). Each core computes its heads'
QKV projections, rope, full attention, and its partial output projection
(rows of Wo for its heads); the host sums the 8 partial outputs (the
all-reduce of the tensor-parallel pattern, done at gather time).

Per-core layouts (everything "transposed", tokens on the free axis):
  xT      [D=2048, B*T=4096]   x transposed (host-prepped), replicated
  wq/wk   [2048, 256]          head-column shard; within each head the 128
                               columns are permuted evens-then-odds so rope
                               pairs become contiguous partition halves
  wv      [2048, 256]          natural column shard
  wo      [256, 2048]          natural row shard
  cs1     [128, 2048]          [cos.T ; sin.T] stacked (64+64 partitions)
  cs2     [128, 2048]          [sin.T ; cos.T]
  ones    [128, 128]           all-ones (softmax denominator matmul)

Pipeline per batch b in {0,1}:
  A) qT/kT = W.T @ xT (per head, [128, 2048]); rope via partition-half
     realign DMAs; v = xT.T @ wv (natural [token,256] tiles).
  B) per (head, i-block of 512): sT[j,i] = kT_slice.T @ qT_slice; e = exp(sT*sc)
     (ACT, psum->sbuf); oT += v_slice.T @ e; denom += ones.T @ e (all-ones
     lhsT broadcasts the row-sum across partitions for free);
     oT_norm = oT * recip_approx(denom).
  C) y[t,d] = sum_h oT_h.T @ wo_h, psum->sbuf, DMA out (partial sums).

All matmul inputs are float32r (full-rate PE, ~1.6e-4 rounding).
"""

import sys

sys.path.insert(0, "/opt/trn_rl_repo")

import numpy as np

import concourse.bass as bass
import concourse.mybir as mybir
import concourse.tile as tile
from concourse import bacc
from concourse.bass_utils import run_bass_kernel_spmd

B, T, D = 2, 2048, 2048
NH, HD = 16, 128
NCORES = 8
HPC = NH // NCORES          # heads per core = 2
CPC = HPC * HD              # proj columns per core = 256
BT = B * T                  # 4096 tokens
P = 128
TBLK = 256                  # phase-A token block
NBLK = T // TBLK            # 8 blocks per batch
DC = D // P                 # 16 contraction chunks
IBLK = 512                  # phase-B query block
NIB = T // IBLK             # 4 i-blocks per batch
NJT = T // P                # 16 key tiles per batch
SCALE = 1.0 / float(np.sqrt(HD))

f32 = mybir.dt.float32
f32r = mybir.dt.float32r

_compiled = {}

# exposed for test.py
last_results = None


def _build():
    nc = bacc.Bacc("TRN2", target_bir_lowering=False, debug=False)

    xT_d = nc.dram_tensor("xT", [D, BT], f32, kind="ExternalInput").ap()
    wq_d = nc.dram_tensor("wq", [D, CPC], f32, kind="ExternalInput").ap()
    wk_d = nc.dram_tensor("wk", [D, CPC], f32, kind="ExternalInput").ap()
    wv_d = nc.dram_tensor("wv", [D, CPC], f32, kind="ExternalInput").ap()
    wo_d = nc.dram_tensor("wo", [CPC, D], f32, kind="ExternalInput").ap()
    cs1_d = nc.dram_tensor("cs1", [P, T], f32, kind="ExternalInput").ap()
    cs2_d = nc.dram_tensor("cs2", [P, T], f32, kind="ExternalInput").ap()
    ones_d = nc.dram_tensor("ones", [P, P], f32, kind="ExternalInput").ap()
    y_d = nc.dram_tensor("y", [BT, D], f32, kind="ExternalOutput").ap()

    with tile.TileContext(nc) as tc:
        _emit(nc, tc, xT_d, wq_d, wk_d, wv_d, wo_d, cs1_d, cs2_d, ones_d, y_d)
    nc.compile()
    return nc


def _emit(nc, tc, xT_d, wq_d, wk_d, wv_d, wo_d, cs1_d, cs2_d, ones_d, y_d):
    from contextlib import ExitStack

    Exp = mybir.ActivationFunctionType.Exp
    mult = mybir.AluOpType.mult
    add = mybir.AluOpType.add
    sub = mybir.AluOpType.subtract

    with ExitStack() as ctx:
        const = ctx.enter_context(tc.tile_pool(name="const", bufs=1))
        state = ctx.enter_context(tc.tile_pool(name="state", bufs=1))

        wq_sb = const.tile([P, DC * CPC], f32r, tag="wq")
        wk_sb = const.tile([P, DC * CPC], f32r, tag="wk")
        wv_sb = const.tile([P, DC * CPC], f32r, tag="wv")
        wo_sb = const.tile([P, HPC * D], f32r, tag="wo")
        cs1_sb = const.tile([P, T], f32, tag="cs1")
        cs2_sb = const.tile([P, T], f32, tag="cs2")
        ones_sb = const.tile([P, P], f32r, tag="ones")

        nc.sync.dma_start(
            wq_sb[:].rearrange("p (dc c) -> p dc c", dc=DC),
            wq_d.rearrange("(dc p) c -> p dc c", p=P).bitcast(f32r))
        nc.sync.dma_start(
            wk_sb[:].rearrange("p (dc c) -> p dc c", dc=DC),
            wk_d.rearrange("(dc p) c -> p dc c", p=P).bitcast(f32r))
        nc.sync.dma_start(
            wv_sb[:].rearrange("p (dc c) -> p dc c", dc=DC),
            wv_d.rearrange("(dc p) c -> p dc c", p=P).bitcast(f32r))
        nc.sync.dma_start(
            wo_sb[:].rearrange("p (h d) -> p h d", h=HPC),
            wo_d.rearrange("(h p) d -> p h d", p=P).bitcast(f32r))
        nc.sync.dma_start(cs1_sb[:], cs1_d[:])
        nc.sync.dma_start(cs2_sb[:], cs2_d[:])
        nc.sync.dma_start(ones_sb[:], ones_d.bitcast(f32r))

        qT_sb = state.tile([P, HPC * T], f32r, tag="qT")
        kT_sb = state.tile([P, HPC * T], f32r, tag="kT")
        v_sb = state.tile([P, NJT * CPC], f32r, tag="v")
        oT_sb = state.tile([P, HPC * T], f32r, tag="oT")

        xpool = ctx.enter_context(tc.tile_pool(name="xa", bufs=2))
        for b in range(B):
            g0 = b * T

            with tc.tile_pool(name=f"ra{b}", bufs=4) as rpool, \
                 tc.tile_pool(name=f"qk_ps{b}", bufs=5, space="PSUM") as qkps, \
                 tc.tile_pool(name=f"v_ps{b}", bufs=3, space="PSUM") as vps:
                for blk in range(NBLK):
                    t0 = blk * TBLK
                    xt = xpool.tile([P, DC * TBLK], f32r, tag="x")
                    nc.sync.dma_start(
                        xt[:].rearrange("p (dc t) -> p dc t", dc=DC),
                        xT_d[:, g0 + t0:g0 + t0 + TBLK]
                        .rearrange("(dc p) t -> p dc t", p=P).bitcast(f32r))

                    for h in range(HPC):
                        for w_sb, dst in ((wq_sb, qT_sb), (wk_sb, kT_sb)):
                            pps = qkps.tile([P, TBLK], f32, tag="qk")
                            for dc in range(DC):
                                nc.tensor.matmul(
                                    pps[:],
                                    w_sb[:, dc * CPC + h * HD:dc * CPC + (h + 1) * HD],
                                    xt[:, dc * TBLK:(dc + 1) * TBLK],
                                    start=(dc == 0), stop=(dc == DC - 1))
                            m1 = rpool.tile([P, TBLK], f32, tag="m1")
                            m3 = rpool.tile([P, TBLK], f32, tag="m3")
                            c1 = cs1_sb[:, t0:t0 + TBLK]
                            c2 = cs2_sb[:, t0:t0 + TBLK]
                            nc.vector.tensor_tensor(m1[:], pps[:], c1, mult)
                            nc.vector.tensor_tensor(m3[:], pps[:], c2, mult)
                            sw = rpool.tile([P, TBLK], f32, tag="sw")
                            nc.sync.dma_start(sw[0:64, :], m1[64:128, :])
                            nc.sync.dma_start(sw[64:128, :], m3[0:64, :])
                            o = dst[:, h * T + t0:h * T + t0 + TBLK]
                            nc.vector.tensor_tensor(
                                o[0:64, :], m1[0:64, :], sw[0:64, :], sub)
                            nc.vector.tensor_tensor(
                                o[64:128, :], m3[64:128, :], sw[64:128, :], add)

                    vp = vps.tile([P, 2 * CPC], f32, tag="v")
                    for tl in range(2):
                        for dc in range(DC):
                            nc.tensor.matmul(
                                vp[:, tl * CPC:(tl + 1) * CPC],
                                xt[:, dc * TBLK + tl * P:dc * TBLK + (tl + 1) * P],
                                wv_sb[:, dc * CPC:(dc + 1) * CPC],
                                start=(dc == 0), stop=(dc == DC - 1))
                    nc.scalar.copy(
                        v_sb[:, (2 * blk) * CPC:(2 * blk + 2) * CPC], vp[:])

            with tc.tile_pool(name=f"e{b}", bufs=3) as epool, \
                 tc.tile_pool(name=f"rc{b}", bufs=2) as rcpool, \
                 tc.tile_pool(name=f"yb{b}", bufs=3) as ypool, \
                 tc.tile_pool(name=f"s_ps{b}", bufs=4, space="PSUM") as sps, \
                 tc.tile_pool(name=f"o_ps{b}", bufs=1, space="PSUM") as ops, \
                 tc.tile_pool(name=f"d_ps{b}", bufs=1, space="PSUM") as dps, \
                 tc.tile_pool(name=f"y_ps{b}", bufs=2, space="PSUM") as yps:
                for ib in range(NIB):
                    i0 = ib * IBLK
                    for h in range(HPC):
                        q_sl = qT_sb[:, h * T + i0:h * T + i0 + IBLK]
                        op = ops.tile([P, IBLK], f32, tag="o")
                        dn = dps.tile([P, IBLK], f32, tag="d")
                        for jt in range(NJT):
                            sp = sps.tile([P, IBLK], f32, tag="s")
                            nc.tensor.matmul(
                                sp[:],
                                kT_sb[:, h * T + jt * P:h * T + (jt + 1) * P],
                                q_sl, start=True, stop=True)
                            e = epool.tile([P, IBLK], f32r, tag="e")
                            nc.scalar.activation(e[:], sp[:], Exp, scale=SCALE)
                            nc.tensor.matmul(
                                op[:],
                                v_sb[:, jt * CPC + h * HD:jt * CPC + (h + 1) * HD],
                                e[:], start=(jt == 0), stop=(jt == NJT - 1))
                            nc.tensor.matmul(
                                dn[:], ones_sb[:], e[:],
                                start=(jt == 0), stop=(jt == NJT - 1))
                        rcp = rcpool.tile([P, IBLK], f32, tag="rc")
                        nc.vector.reciprocal_approx_fast(out=rcp[:], in_=dn[:])
                        nc.vector.tensor_tensor(
                            oT_sb[:, h * T + i0:h * T + i0 + IBLK],
                            op[:], rcp[:], mult)
                    # output projection for this 512-token block: both heads'
                    # oT are ready, so C's DMA hides under the next ib's
                    # attention instead of a batch-end tail.
                    for tl in range(IBLK // P):
                        tt = ib * (IBLK // P) + tl
                        for db in range(D // IBLK):
                            yp = yps.tile([P, IBLK], f32, tag="y")
                            for h in range(HPC):
                                nc.tensor.matmul(
                                    yp[:],
                                    oT_sb[:, h * T + tt * P:h * T + (tt + 1) * P],
                                    wo_sb[:, h * D + db * IBLK:h * D + (db + 1) * IBLK],
                                    start=(h == 0), stop=(h == HPC - 1))
                            yt = ypool.tile([P, IBLK], f32, tag="yt")
                            nc.scalar.copy(yt[:], yp[:])
                            nc.sync.dma_start(
                                y_d[g0 + tt * P:g0 + (tt + 1) * P,
                                    db * IBLK:(db + 1) * IBLK],
                                yt[:])




_EVEN_ODD = np.concatenate([np.arange(0, HD, 2), np.arange(1, HD, 2)])


def _prep_inputs(x, rope_cos, rope_sin, Wq, Wk, Wv, Wo):
    x = np.asarray(x, dtype=np.float32)
    xT = np.ascontiguousarray(x.reshape(BT, D).T)
    cosT = np.asarray(rope_cos, dtype=np.float32).T
    sinT = np.asarray(rope_sin, dtype=np.float32).T
    cs1 = np.ascontiguousarray(
        np.concatenate([cosT, sinT], axis=0), dtype=np.float32)
    cs2 = np.ascontiguousarray(
        np.concatenate([sinT, cosT], axis=0), dtype=np.float32)
    ones = np.ones((P, P), dtype=np.float32)
    Wq = np.asarray(Wq, dtype=np.float32)
    Wk = np.asarray(Wk, dtype=np.float32)
    Wv = np.asarray(Wv, dtype=np.float32)
    Wo = np.asarray(Wo, dtype=np.float32)

    in_maps = []
    for c in range(NCORES):
        cols = slice(c * CPC, (c + 1) * CPC)
        wq_c = Wq[:, cols].reshape(D, HPC, HD)[:, :, _EVEN_ODD].reshape(D, CPC)
        wk_c = Wk[:, cols].reshape(D, HPC, HD)[:, :, _EVEN_ODD].reshape(D, CPC)
        in_maps.append({
            "xT": xT,
            "wq": np.ascontiguousarray(wq_c),
            "wk": np.ascontiguousarray(wk_c),
            "wv": np.ascontiguousarray(Wv[:, cols]),
            "wo": np.ascontiguousarray(Wo[cols, :]),
            "cs1": cs1,
            "cs2": cs2,
            "ones": ones,
        })
    return in_maps


def kernel(x, rope_cos, rope_sin, Wq, Wk, Wv, Wo, _trace=False):
    global last_results
    if "nc" not in _compiled:
        _compiled["nc"] = _build()
    nc = _compiled["nc"]
    in_maps = _prep_inputs(x, rope_cos, rope_sin, Wq, Wk, Wv, Wo)
    res = run_bass_kernel_spmd(
        nc, in_maps, core_ids=list(range(NCORES)), trace=_trace)
    last_results = res
    y = np.sum(np.stack([res.results[c]["y"] for c in range(NCORES)]),
               axis=0, dtype=np.float64)
    return y.reshape(B, T, D).astype(np.float32)



# revision 3
# speedup vs baseline: 1.2242x; 1.2242x over previous
"""Tensor-parallel causal-self-attention (full-attention) Bass kernel for TRN2.

Sharding: 16 heads over 8 cores (2 heads/core). Each core computes its heads'
QKV projections, rope, full attention, and its partial output projection
(rows of Wo for its heads); the host sums the 8 partial outputs (the
all-reduce of the tensor-parallel pattern, done at gather time).

Per-core layouts (everything "transposed", tokens on the free axis). All
matmul operands are bf16 (same PE rate as f32r, half the DMA/SBUF), psum f32:
  xT      [D=2048, B*T=4096]   x transposed (host-prepped, bf16), replicated
  wq/wk   [2048, 256]          head-column shard; within each head the 128
                               columns are permuted evens-then-odds so rope
                               pairs become contiguous partition halves
  wv      [2048, 256]          natural column shard
  wo      [256, 2048]          natural row shard
  cs1     [128, 2048]          [cos.T ; sin.T] stacked (64+64 partitions), f32
  cs2     [128, 2048]          [sin.T ; cos.T]
  ones    [128, 128]           all-ones (softmax denominator broadcast matmul)

Initial DMAs are issued fine-grained in priority order (wq, x blk0, wk, wv,
cs, wo) so the first matmul starts ~5us in instead of waiting ~45us for a
monolithic const load.

Pipeline per batch b in {0,1}:
  A) qT/kT = W.T @ xT (per head, [128, 512]-token blocks, N=512 matmuls);
     rope via partition-half realign DMAs; v = xT.T @ wv (natural
     [token,256] tiles).
  B) per (head, i-block of 512): score PAIRS into a 2-bank psum tile
     ([128,1024]); ONE exp ACT per pair (psum->sbuf bf16); oT += v.T @ e.
     Softmax denominator: DVE accumulates the e-pairs (bf16 2x) and a single
     ones-matmul pair broadcasts the partition-sum — replaces 16 full
     denominator matmuls per (h,ib) with 2 (saves ~50us of PE streaming).
     oT_norm = op * recip_approx(denom).
  C) y[t,d] = sum_h oT_h.T @ wo_h per i-block; psum->sbuf staging copies run
     on gpsimd/vector (scalar stays exp-only), DMA out (partial sums).
"""

import sys

sys.path.insert(0, "/opt/trn_rl_repo")

import numpy as np
import ml_dtypes

import concourse.bass as bass
import concourse.mybir as mybir
import concourse.tile as tile
from concourse import bacc
from concourse.bass_utils import run_bass_kernel_spmd

B, T, D = 2, 2048, 2048
NH, HD = 16, 128
NCORES = 8
HPC = NH // NCORES          # heads per core = 2
CPC = HPC * HD              # proj columns per core = 256
BT = B * T                  # 4096 tokens
P = 128
TBLK = 512                  # phase-A token block
NBLK = T // TBLK            # 4 blocks per batch
DC = D // P                 # 16 contraction chunks
IBLK = 512                  # phase-B query block
NIB = T // IBLK             # 4 i-blocks per batch
NJT = T // P                # 16 key tiles per batch
NPR = NJT // 2              # 8 key-tile pairs
SCALE = 1.0 / float(np.sqrt(HD))

f32 = mybir.dt.float32
bf16 = mybir.dt.bfloat16

_compiled = {}

# exposed for test.py
last_results = None


def _build():
    nc = bacc.Bacc("TRN2", target_bir_lowering=False, debug=False)

    xT_d = nc.dram_tensor("xT", [D, BT], bf16, kind="ExternalInput").ap()
    wq_d = nc.dram_tensor("wq", [D, CPC], bf16, kind="ExternalInput").ap()
    wk_d = nc.dram_tensor("wk", [D, CPC], bf16, kind="ExternalInput").ap()
    wv_d = nc.dram_tensor("wv", [D, CPC], bf16, kind="ExternalInput").ap()
    wo_d = nc.dram_tensor("wo", [CPC, D], bf16, kind="ExternalInput").ap()
    cs1_d = nc.dram_tensor("cs1", [P, T], f32, kind="ExternalInput").ap()
    cs2_d = nc.dram_tensor("cs2", [P, T], f32, kind="ExternalInput").ap()
    ones_d = nc.dram_tensor("ones", [P, P], bf16, kind="ExternalInput").ap()
    y_d = nc.dram_tensor("y", [BT, D], f32, kind="ExternalOutput").ap()

    with tile.TileContext(nc) as tc:
        _emit(nc, tc, xT_d, wq_d, wk_d, wv_d, wo_d, cs1_d, cs2_d, ones_d, y_d)
    nc.compile()
    return nc


def _emit(nc, tc, xT_d, wq_d, wk_d, wv_d, wo_d, cs1_d, cs2_d, ones_d, y_d):
    from contextlib import ExitStack

    Exp = mybir.ActivationFunctionType.Exp
    mult = mybir.AluOpType.mult
    add = mybir.AluOpType.add
    sub = mybir.AluOpType.subtract

    with ExitStack() as ctx:
        const = ctx.enter_context(tc.tile_pool(name="const", bufs=1))
        state = ctx.enter_context(tc.tile_pool(name="state", bufs=1))
        xpool = ctx.enter_context(tc.tile_pool(name="xa", bufs=4))
        rpool = ctx.enter_context(tc.tile_pool(name="ra", bufs=4))
        epool = ctx.enter_context(tc.tile_pool(name="e", bufs=3))
        accpool = ctx.enter_context(tc.tile_pool(name="ac", bufs=2))
        rcpool = ctx.enter_context(tc.tile_pool(name="rc", bufs=2))
        ypool = ctx.enter_context(tc.tile_pool(name="yb", bufs=4))

        wq_sb = const.tile([P, DC * CPC], bf16, tag="wq")
        wk_sb = const.tile([P, DC * CPC], bf16, tag="wk")
        wv_sb = const.tile([P, DC * CPC], bf16, tag="wv")
        wo_sb = const.tile([P, HPC * D], bf16, tag="wo")
        cs1_sb = const.tile([P, T], f32, tag="cs1")
        cs2_sb = const.tile([P, T], f32, tag="cs2")
        ones_sb = const.tile([P, P], bf16, tag="ones")

        qT_sb = state.tile([P, HPC * T], bf16, tag="qT")
        kT_sb = state.tile([P, HPC * T], bf16, tag="kT")
        v_sb = state.tile([P, NJT * CPC], bf16, tag="v")
        oT_sb = state.tile([P, HPC * T], bf16, tag="oT")

        def load_w(sb, dr, c0, c1):
            # dc-chunk [c0, c1) of a [D, CPC] weight into [P, DC*CPC] sbuf
            nc.sync.dma_start(
                sb[:, c0 * CPC:c1 * CPC]
                .rearrange("p (dc c) -> p dc c", dc=c1 - c0),
                dr[c0 * P:c1 * P, :].rearrange("(dc p) c -> p dc c", p=P))

        xtiles = {}

        def load_x(b, blk):
            t0 = b * T + blk * TBLK
            xt = xpool.tile([P, DC * TBLK], bf16, tag="x")
            for c0, c1 in ((0, DC // 2), (DC // 2, DC)):
                nc.sync.dma_start(
                    xt[:, c0 * TBLK:c1 * TBLK]
                    .rearrange("p (dc t) -> p dc t", dc=c1 - c0),
                    xT_d[c0 * P:c1 * P, t0:t0 + TBLK]
                    .rearrange("(dc p) t -> p dc t", p=P))
            return xt

        # priority-ordered startup: what the first matmuls need goes first
        load_w(wq_sb, wq_d, 0, DC // 2)
        load_w(wq_sb, wq_d, DC // 2, DC)
        xtiles[(0, 0)] = load_x(0, 0)
        load_w(wk_sb, wk_d, 0, DC // 2)
        load_w(wk_sb, wk_d, DC // 2, DC)
        load_w(wv_sb, wv_d, 0, DC // 2)
        load_w(wv_sb, wv_d, DC // 2, DC)
        nc.sync.dma_start(cs1_sb[:], cs1_d[:])
        nc.sync.dma_start(cs2_sb[:], cs2_d[:])
        nc.sync.dma_start(ones_sb[:], ones_d[:])
        for h in range(HPC):  # wo only needed at first outproj (~90us in)
            nc.sync.dma_start(wo_sb[:, h * D:(h + 1) * D],
                              wo_d[h * P:(h + 1) * P, :])

        for b in range(B):
            g0 = b * T

            with tc.tile_pool(name=f"qk_ps{b}", bufs=5, space="PSUM") as qkps, \
                 tc.tile_pool(name=f"v_ps{b}", bufs=1, space="PSUM") as vps:
                for blk in range(NBLK):
                    t0 = blk * TBLK
                    xt = xtiles.pop((b, blk), None)
                    if xt is None:
                        xt = load_x(b, blk)

                    for h in range(HPC):
                        for w_sb, dst in ((wq_sb, qT_sb), (wk_sb, kT_sb)):
                            pps = qkps.tile([P, TBLK], f32, tag="qk")
                            for dc in range(DC):
                                nc.tensor.matmul(
                                    pps[:],
                                    w_sb[:, dc * CPC + h * HD:dc * CPC + (h + 1) * HD],
                                    xt[:, dc * TBLK:(dc + 1) * TBLK],
                                    start=(dc == 0), stop=(dc == DC - 1))
                            m1 = rpool.tile([P, TBLK], f32, tag="m1")
                            m3 = rpool.tile([P, TBLK], f32, tag="m3")
                            c1 = cs1_sb[:, t0:t0 + TBLK]
                            c2 = cs2_sb[:, t0:t0 + TBLK]
                            nc.vector.tensor_tensor(m1[:], pps[:], c1, mult)
                            nc.vector.tensor_tensor(m3[:], pps[:], c2, mult)
                            sw = rpool.tile([P, TBLK], f32, tag="sw")
                            nc.sync.dma_start(sw[0:64, :], m1[64:128, :])
                            nc.sync.dma_start(sw[64:128, :], m3[0:64, :])
                            o = dst[:, h * T + t0:h * T + t0 + TBLK]
                            nc.vector.tensor_tensor(
                                o[0:64, :], m1[0:64, :], sw[0:64, :], sub)
                            nc.vector.tensor_tensor(
                                o[64:128, :], m3[64:128, :], sw[64:128, :], add)

                    vp = vps.tile([P, 4 * CPC], f32, tag="v")
                    for tl in range(TBLK // P):
                        for dc in range(DC):
                            nc.tensor.matmul(
                                vp[:, tl * CPC:(tl + 1) * CPC],
                                xt[:, dc * TBLK + tl * P:dc * TBLK + (tl + 1) * P],
                                wv_sb[:, dc * CPC:(dc + 1) * CPC],
                                start=(dc == 0), stop=(dc == DC - 1))
                    nc.scalar.copy(
                        v_sb[:, blk * 4 * CPC:(blk + 1) * 4 * CPC], vp[:])

            # prefetch next batch's x during this batch's attention
            if b + 1 < B:
                for nblk in range(NBLK):
                    xtiles[(b + 1, nblk)] = load_x(b + 1, nblk)

            with tc.tile_pool(name=f"s_ps{b}", bufs=2, space="PSUM") as sps, \
                 tc.tile_pool(name=f"o_ps{b}", bufs=1, space="PSUM") as ops, \
                 tc.tile_pool(name=f"d_ps{b}", bufs=1, space="PSUM") as dps, \
                 tc.tile_pool(name=f"y_ps{b}", bufs=2, space="PSUM") as yps:
                for ib in range(NIB):
                    i0 = ib * IBLK
                    for h in range(HPC):
                        q_sl = qT_sb[:, h * T + i0:h * T + i0 + IBLK]
                        op = ops.tile([P, IBLK], f32, tag="o")
                        acc = accpool.tile([P, 2 * IBLK], bf16, tag="acc")
                        for pr in range(NPR):
                            j0, j1 = 2 * pr, 2 * pr + 1
                            sp = sps.tile([P, 2 * IBLK], f32, tag="s")
                            nc.tensor.matmul(
                                sp[:, 0:IBLK],
                                kT_sb[:, h * T + j0 * P:h * T + (j0 + 1) * P],
                                q_sl, start=True, stop=True)
                            nc.tensor.matmul(
                                sp[:, IBLK:2 * IBLK],
                                kT_sb[:, h * T + j1 * P:h * T + (j1 + 1) * P],
                                q_sl, start=True, stop=True)
                            e = epool.tile([P, 2 * IBLK], bf16, tag="e")
                            nc.scalar.activation(e[:], sp[:], Exp, scale=SCALE)
                            nc.tensor.matmul(
                                op[:],
                                v_sb[:, j0 * CPC + h * HD:j0 * CPC + (h + 1) * HD],
                                e[:, 0:IBLK], start=(pr == 0), stop=False)
                            nc.tensor.matmul(
                                op[:],
                                v_sb[:, j1 * CPC + h * HD:j1 * CPC + (h + 1) * HD],
                                e[:, IBLK:2 * IBLK],
                                start=False, stop=(pr == NPR - 1))
                            if pr == 0:
                                nc.vector.tensor_scalar_mul(acc[:], e[:], 1.0)
                            else:
                                nc.vector.tensor_tensor(acc[:], acc[:], e[:], add)
                        dn = dps.tile([P, IBLK], f32, tag="d")
                        nc.tensor.matmul(dn[:], ones_sb[:], acc[:, 0:IBLK],
                                         start=True, stop=False)
                        nc.tensor.matmul(dn[:], ones_sb[:], acc[:, IBLK:2 * IBLK],
                                         start=False, stop=True)
                        rcp = rcpool.tile([P, IBLK], f32, tag="rc")
                        nc.vector.reciprocal_approx_fast(out=rcp[:], in_=dn[:])
                        nc.vector.tensor_tensor(
                            oT_sb[:, h * T + i0:h * T + i0 + IBLK],
                            op[:], rcp[:], mult)
                    # output projection for this 512-token block hides under
                    # the next ib's attention; staging copies on gpsimd/vector
                    # keep the scalar engine exp-only.
                    for tl in range(IBLK // P):
                        tt = ib * (IBLK // P) + tl
                        for db in range(D // IBLK):
                            yp = yps.tile([P, IBLK], f32, tag="y")
                            for h in range(HPC):
                                nc.tensor.matmul(
                                    yp[:],
                                    oT_sb[:, h * T + tt * P:h * T + (tt + 1) * P],
                                    wo_sb[:, h * D + db * IBLK:h * D + (db + 1) * IBLK],
                                    start=(h == 0), stop=(h == HPC - 1))
                            yt = ypool.tile([P, IBLK], f32, tag="yt")
                            # gpsimd cannot read PSUM; split staging copies
                            # scalar/vector to keep both under the PE window
                            if (tl * (D // IBLK) + db) % 8 < 3:
                                nc.scalar.copy(yt[:], yp[:])
                            else:
                                nc.vector.tensor_scalar_mul(yt[:], yp[:], 1.0)
                            nc.sync.dma_start(
                                y_d[g0 + tt * P:g0 + (tt + 1) * P,
                                    db * IBLK:(db + 1) * IBLK],
                                yt[:])


_EVEN_ODD = np.concatenate([np.arange(0, HD, 2), np.arange(1, HD, 2)])
_BF16 = ml_dtypes.bfloat16


def _prep_inputs(x, rope_cos, rope_sin, Wq, Wk, Wv, Wo):
    x = np.asarray(x, dtype=np.float32)
    xT = np.ascontiguousarray(x.reshape(BT, D).T.astype(_BF16))
    cosT = np.asarray(rope_cos, dtype=np.float32).T
    sinT = np.asarray(rope_sin, dtype=np.float32).T
    cs1 = np.ascontiguousarray(
        np.concatenate([cosT, sinT], axis=0), dtype=np.float32)
    cs2 = np.ascontiguousarray(
        np.concatenate([sinT, cosT], axis=0), dtype=np.float32)
    ones = np.ones((P, P), dtype=_BF16)
    Wq = np.asarray(Wq, dtype=np.float32)
    Wk = np.asarray(Wk, dtype=np.float32)
    Wv = np.asarray(Wv, dtype=np.float32)
    Wo = np.asarray(Wo, dtype=np.float32)

    in_maps = []
    for c in range(NCORES):
        cols = slice(c * CPC, (c + 1) * CPC)
        wq_c = Wq[:, cols].reshape(D, HPC, HD)[:, :, _EVEN_ODD].reshape(D, CPC)
        wk_c = Wk[:, cols].reshape(D, HPC, HD)[:, :, _EVEN_ODD].reshape(D, CPC)
        in_maps.append({
            "xT": xT,
            "wq": np.ascontiguousarray(wq_c.astype(_BF16)),
            "wk": np.ascontiguousarray(wk_c.astype(_BF16)),
            "wv": np.ascontiguousarray(Wv[:, cols].astype(_BF16)),
            "wo": np.ascontiguousarray(Wo[cols, :].astype(_BF16)),
            "cs1": cs1,
            "cs2": cs2,
            "ones": ones,
        })
    return in_maps


def kernel(x, rope_cos, rope_sin, Wq, Wk, Wv, Wo, _trace=False):
    global last_results
    if "nc" not in _compiled:
        _compiled["nc"] = _build()
    nc = _compiled["nc"]
    in_maps = _prep_inputs(x, rope_cos, rope_sin, Wq, Wk, Wv, Wo)
    res = run_bass_kernel_spmd(
        nc, in_maps, core_ids=list(range(NCORES)), trace=_trace)
    last_results = res
    y = np.sum(np.stack([res.results[c]["y"] for c in range(NCORES)]),
               axis=0, dtype=np.float64)
    return y.reshape(B, T, D).astype(np.float32)
